# revision 11
# baseline (speedup 1.0000x reference)
"""Multi-headed attention (B=4, S=2048, D=1024, H=16) on 8 trn2 NeuronCores.

Sharding: core c handles batch b=c//2, head-half hh=c%2 (heads hh*8..hh*8+7).

v2: single fused pipeline (no phase barrier), all inputs pre-cast to bf16 on
the host (no on-device casts), V bias applied on DVE during drain (no K=1
bias matmuls), and background PE work (V projection, previous-tile output
projection) interleaved into the scores stream so the ACT engine (softmax
exp, the critical resource at ~285us) never starves.

Per core:
  K projection (feature-major [512, 2048]) emitted first; then per query
  tile t (512 queries) and head-pair j: Q projection, scores_T via paired
  K=64 matmuls on PE row halves, exp on ACT (1/8 scale folded), AV with
  ones-augmented V accumulating unnormalized X + row sums.  V-projection
  groups are pumped into the (t=0, j=0) scores stream; output-projection
  groups for tile t-1 are pumped into tile t's scores stream.
Host: out[b] = core(2b) + core(2b+1) + bo.
"""

import numpy as np
import ml_dtypes

import concourse.tile as tile
from concourse import bacc, mybir
from concourse.bass_utils import run_bass_kernel_spmd

B, S, D, H = 4, 2048, 1024, 16
HD = D // 2          # feature columns per core (8 heads * 64)
KC = D // 128        # 8 contraction chunks over model dim
FT = HD // 128       # 4 feature tiles (head pairs)
ST = S // 512        # 4 query tiles
RT = S // 128        # 16 row tiles / S_k chunks

f32 = mybir.dt.float32
bf16 = mybir.dt.bfloat16
MM_DT = bf16
EXP = mybir.ActivationFunctionType.Exp

_CACHED_NC = None
_LAST_IN_MAPS = None


def build_nc():
    nc = bacc.Bacc("TRN2", target_bir_lowering=False, debug=False)

    xq_d = nc.dram_tensor("xq", (D, S), bf16, kind="ExternalInput")
    xk_d = nc.dram_tensor("xk", (D, S), bf16, kind="ExternalInput")
    xv_d = nc.dram_tensor("xv", (D, S), bf16, kind="ExternalInput")
    wq_d = nc.dram_tensor("wq", (D, HD), bf16, kind="ExternalInput")
    wk_d = nc.dram_tensor("wk", (D, HD), bf16, kind="ExternalInput")
    wv_d = nc.dram_tensor("wv", (D, HD), bf16, kind="ExternalInput")
    wo_d = nc.dram_tensor("wo", (HD, D), bf16, kind="ExternalInput")
    bqr_d = nc.dram_tensor("bqr", (128, FT), f32, kind="ExternalInput")
    bkr_d = nc.dram_tensor("bkr", (128, FT), f32, kind="ExternalInput")
    bv_d = nc.dram_tensor("bv", (1, HD), f32, kind="ExternalInput")
    o_d = nc.dram_tensor("o", (S, D), f32, kind="ExternalOutput")

    with tile.TileContext(nc) as tc:
        with (
            tc.tile_pool(name="cpool", bufs=1) as cpool,
            tc.tile_pool(name="big", bufs=1) as big,
            tc.tile_pool(name="xs", bufs=16) as xsp,
            tc.tile_pool(name="xqp", bufs=16) as xqp,
            tc.tile_pool(name="qt", bufs=3) as qtp,
            tc.tile_pool(name="ptp", bufs=16) as ptp,
            tc.tile_pool(name="nrm", bufs=4) as nrm,
            tc.tile_pool(name="rsp", bufs=2) as rsp,
            tc.tile_pool(name="ostage", bufs=4) as ostage,
            tc.tile_pool(name="rsd", bufs=2, space="DRAM") as rsd,
            tc.tile_pool(name="psc", bufs=2, space="PSUM") as psc,
            tc.tile_pool(name="px", bufs=2, space="PSUM") as px,
            tc.tile_pool(name="pq", bufs=2, space="PSUM") as pq,
        ):
            # ---------------- constants / biases ----------------
            onecol_f = cpool.tile([128, 1], f32, name="onecol_f")
            nc.gpsimd.memset(onecol_f[:], 1.0)
            bqr_s = cpool.tile([128, FT], f32, name="bqr_s")
            nc.sync.dma_start(bqr_s[:], bqr_d[:])
            bkr_s = cpool.tile([128, FT], f32, name="bkr_s")
            nc.sync.dma_start(bkr_s[:], bkr_d[:])
            bv_bc = cpool.tile([128, HD], f32, name="bv_bc")
            nc.sync.dma_start(bv_bc[:], bv_d[0:1, :].to_broadcast((128, HD)))

            K = big.tile([128, FT, S], MM_DT, name="Kfm")
            Vs = big.tile([128, RT, 8, 65], MM_DT, name="Vs")
            X = big.tile([128, FT, S], MM_DT, name="Xfm")
            nc.vector.tensor_copy(
                Vs[:, :, :, 64:65],
                onecol_f[:, 0:1].to_broadcast((128, RT, 8, 1)),
            )

            # ---------------- weights: direct bf16 DMA ----------------
            wk_s = big.tile([128, KC, HD], MM_DT, name="wk_s")
            wv_s = big.tile([128, KC, HD], MM_DT, name="wv_s")
            wq_s = big.tile([128, KC, HD], MM_DT, name="wq_s")
            wo_s = big.tile([128, FT, D], MM_DT, name="wo_s")
            wk_src = wk_d[:].rearrange("(k p) n -> p k n", p=128)
            wv_src = wv_d[:].rearrange("(k p) n -> p k n", p=128)
            wq_src = wq_d[:].rearrange("(k p) n -> p k n", p=128)
            wo_src = wo_d[:].rearrange("(k p) n -> p k n", p=128)
            for kc in range(KC):
                nc.sync.dma_start(wk_s[:, kc, :], wk_src[:, kc, :])
            for kc in range(KC):
                nc.gpsimd.dma_start(wv_s[:, kc, :], wv_src[:, kc, :])
            for kc in range(KC):
                nc.gpsimd.dma_start(wq_s[:, kc, :], wq_src[:, kc, :])
            for fc in range(FT):
                nc.gpsimd.dma_start(wo_s[:, fc, :], wo_src[:, fc, :])

            def load_x_tiles(x_d, t, eng, tag):
                ts = []
                for kc in range(KC):
                    xt = xsp.tile([128, 512], MM_DT, tag=tag, name=tag)
                    eng.dma_start(
                        xt[:], x_d[kc * 128 : (kc + 1) * 128,
                                   t * 512 : (t + 1) * 512])
                    ts.append(xt)
                return ts

            # ---------------- K projection (feature-major) ----------------
            nxt = load_x_tiles(xk_d, 0, nc.sync, "xs")
            xq_tiles = {}

            def load_xq(t):
                xq_tiles[t] = []
                for kc in range(KC):
                    xt = xqp.tile([128, 512], MM_DT, tag="xq", name="xq")
                    nc.gpsimd.dma_start(
                        xt[:], xq_d[kc * 128 : (kc + 1) * 128,
                                    t * 512 : (t + 1) * 512])
                    xq_tiles[t].append(xt)

            load_xq(0)
            for t in range(ST):
                cur = nxt
                if t + 1 < ST:
                    nxt = load_x_tiles(xk_d, t + 1, nc.sync, "xs")
                for ft in range(FT):
                    ps = pq.tile([128, 512], f32, tag="pacc", name="pk")
                    for kc in range(KC):
                        nc.tensor.matmul(
                            ps[:],
                            wk_s[:, kc, ft * 128 : (ft + 1) * 128],
                            cur[kc][:],
                            start=(kc == 0),
                            stop=(kc == KC - 1),
                        )
                    nc.vector.tensor_scalar_add(
                        K[:, ft, t * 512 : (t + 1) * 512],
                        ps[:],
                        bkr_s[:, ft : ft + 1],
                    )

            # ---------------- deferred V projection groups ----------------
            xv_tiles = {}

            def load_xv(g):
                xv_tiles[g] = load_x_tiles(xv_d, g, nc.gpsimd, "xs")

            load_xv(0)
            vstate = [0]

            def emit_vgroup():
                rt = vstate[0]
                g, rr = rt // 4, rt % 4
                if rr == 0 and g + 1 < 4:
                    load_xv(g + 1)
                ps = pq.tile([128, 512], f32, tag="pacc", name="pv")
                for kc in range(KC):
                    nc.tensor.matmul(
                        ps[:],
                        xv_tiles[g][kc][:, rr * 128 : (rr + 1) * 128],
                        wv_s[:, kc, :],
                        start=(kc == 0),
                        stop=(kc == KC - 1),
                    )
                nc.vector.tensor_add(
                    Vs[:, rt, :, 0:64],
                    ps[:].rearrange("p (h e) -> p h e", h=8),
                    bv_bc[:].rearrange("p (h e) -> p h e", h=8),
                )
                vstate[0] = rt + 1

            def pump_v(n):
                for _ in range(n):
                    if vstate[0] < RT:
                        emit_vgroup()

            # ---------------- deferred output projection ----------------
            bg = []

            def mk_outproj(t2):
                def mk(r2, n):
                    def g():
                        rt = t2 * 4 + r2
                        rsl = slice(rt * 128, (rt + 1) * 128)
                        ps = pq.tile([128, 512], f32, tag="pacc", name="pso")
                        for fc in range(FT):
                            nc.tensor.matmul(
                                ps[:],
                                X[:, fc, rsl],
                                wo_s[:, fc, n * 512 : (n + 1) * 512],
                                start=(fc == 0),
                                stop=(fc == FT - 1),
                            )
                        ot = ostage.tile([128, 512], f32, tag="os", name="os")
                        nc.vector.tensor_copy(ot[:], ps[:])
                        nc.gpsimd.dma_start(
                            o_d[rsl, n * 512 : (n + 1) * 512], ot[:])
                    return g
                return [mk(r2, n) for r2 in range(4) for n in range(2)]

            def pump_bg(n):
                for _ in range(n):
                    if bg:
                        bg.pop(0)()

            # ---------------- normalization ----------------
            def emit_normalize(j2, rsj, tsl2):
                rrh = nrm.tile([128, 512], f32, tag="rr", name="rr")
                nc.vector.reciprocal_approx_fast(rrh[:], rsj[:])
                rd = rsd.tile([2, 512], f32, tag="rd", name="rd")
                for hh in range(2):
                    nc.sync.dma_start(
                        rd[hh : hh + 1, :],
                        rrh[32 * hh : 32 * hh + 1, :])
                for hh in range(2):
                    pb = 64 * hh
                    bcs = nrm.tile([128, 512], f32, tag="bcs", name="bcs")
                    nc.sync.dma_start(
                        bcs[pb : pb + 64, :],
                        rd[hh : hh + 1, :].to_broadcast((64, 512)))
                    nc.vector.tensor_mul(
                        X[pb : pb + 64, j2, tsl2],
                        X[pb : pb + 64, j2, tsl2],
                        bcs[pb : pb + 64, :],
                    )

            # ---------------- attention ----------------
            def emit_av(item, xpA, xpB, j):
                cc, pA, pB = item
                for hf in range(2):
                    kc = 2 * cc + hf
                    nc.tensor.matmul(
                        xpA[:], Vs[:, kc, 2 * j, :], pA[:, hf, :],
                        start=(kc == 0), stop=(kc == RT - 1),
                    )
                    nc.tensor.matmul(
                        xpB[:], Vs[:, kc, 2 * j + 1, :], pB[:, hf, :],
                        start=(kc == 0), stop=(kc == RT - 1),
                    )

            norm_pending = None
            for t in range(ST):
                tsl = slice(t * 512, (t + 1) * 512)
                for j in range(FT):
                    first = (t == 0 and j == 0)
                    # Q projection for this pair
                    qp = pq.tile([128, 512], f32, tag="pacc", name="qp")
                    for kc in range(KC):
                        nc.tensor.matmul(
                            qp[:],
                            wq_s[:, kc, j * 128 : (j + 1) * 128],
                            xq_tiles[t][kc][:],
                            start=(kc == 0),
                            stop=(kc == KC - 1),
                        )
                    Qt = qtp.tile([128, 512], MM_DT, tag="qt", name="qt")
                    nc.vector.tensor_scalar_add(
                        Qt[:], qp[:], bqr_s[:, j : j + 1])
                    if norm_pending is not None:
                        emit_normalize(*norm_pending)
                        norm_pending = None
                    if j == 2 and t + 1 < ST:
                        load_xq(t + 1)

                    xpA = px.tile([65, 512], f32, tag="px", name="xpA")
                    xpB = px.tile([65, 512], f32, tag="px", name="xpB")
                    pend = []
                    for cc in range(8):
                        sA = psc.tile([128, 2, 512], f32, tag="sc", name="sA")
                        sB = psc.tile([128, 2, 512], f32, tag="sc", name="sB")
                        for hf in range(2):
                            kc = 2 * cc + hf
                            ksl = slice(kc * 128, (kc + 1) * 128)
                            nc.tensor.matmul(
                                sA[:, hf, :], K[0:64, j, ksl], Qt[0:64, :],
                                start=True, stop=True, tile_position=(0, 0),
                            )
                            nc.tensor.matmul(
                                sB[:, hf, :], K[64:128, j, ksl],
                                Qt[64:128, :],
                                start=True, stop=True, tile_position=(64, 0),
                            )
                        pA = ptp.tile([128, 2, 512], MM_DT, tag="pt",
                                      name="pA")
                        nc.scalar.activation(pA[:], sA[:], EXP, scale=0.125)
                        pB = ptp.tile([128, 2, 512], MM_DT, tag="pt",
                                      name="pB")
                        nc.scalar.activation(pB[:], sB[:], EXP, scale=0.125)
                        pend.append((cc, pA, pB))
                        if first:
                            # build V while scores of the first pair stream
                            pump_v(2 if cc < 4 else 1)
                            # start AV for chunks whose V rows are ready
                            while (pend and
                                   2 * pend[0][0] + 1 < vstate[0] - 2):
                                emit_av(pend.pop(0), xpA, xpB, j)
                        else:
                            if len(pend) > 2:
                                emit_av(pend.pop(0), xpA, xpB, j)
                            if cc in (2, 4, 6):
                                pump_bg(1)
                    for item in pend:
                        if first:
                            while vstate[0] <= 2 * item[0] + 1:
                                emit_vgroup()
                        emit_av(item, xpA, xpB, j)

                    # drain: unnormalized X and row sums to SBUF
                    nc.vector.tensor_copy(X[0:64, j, tsl], xpA[0:64, :])
                    nc.vector.tensor_copy(X[64:128, j, tsl], xpB[0:64, :])
                    rsj = rsp.tile([128, 512], f32, tag="rs", name="rs")
                    nc.vector.tensor_copy(rsj[0:1, :], xpA[64:65, :])
                    nc.vector.tensor_copy(rsj[32:33, :], xpB[64:65, :])
                    norm_pending = (j, rsj, tsl)

                # end of tile t: queue its output projection for overlap
                if t == ST - 1:
                    if norm_pending is not None:
                        emit_normalize(*norm_pending)
                        norm_pending = None
                    pump_bg(len(bg))
                    for g in mk_outproj(t):
                        g()
                else:
                    bg.extend(mk_outproj(t))

    nc.compile()
    return nc


def kernel(**inputs):
    global _CACHED_NC, _LAST_IN_MAPS
    if _CACHED_NC is None:
        _CACHED_NC = build_nc()
    nc = _CACHED_NC

    bfdt = ml_dtypes.bfloat16
    query = np.asarray(inputs["query"], dtype=np.float32)
    key = np.asarray(inputs["key"], dtype=np.float32)
    value = np.asarray(inputs["value"], dtype=np.float32)
    fc_w = np.asarray(inputs["fc_w"], dtype=np.float32)
    Wq = np.asarray(inputs["Wq"], dtype=np.float32)
    Wk = np.asarray(inputs["Wk"], dtype=np.float32)
    Wv = np.asarray(inputs["Wv"], dtype=np.float32)
    Wo = np.asarray(inputs["Wo"], dtype=np.float32)
    bq = np.asarray(inputs["bq"], dtype=np.float32)
    bk = np.asarray(inputs["bk"], dtype=np.float32)
    bv = np.asarray(inputs["bv"], dtype=np.float32)
    bo = np.asarray(inputs["bo"], dtype=np.float32)

    wq_eff = (fc_w * Wq).astype(bfdt)
    wk_b = Wk.astype(bfdt)
    wv_b = Wv.astype(bfdt)
    wo_b = Wo.astype(bfdt)
    xq_b = [np.ascontiguousarray(query[b].T).astype(bfdt) for b in range(B)]
    xk_b = [np.ascontiguousarray(key[b].T).astype(bfdt) for b in range(B)]
    xv_b = [np.ascontiguousarray(value[b].T).astype(bfdt) for b in range(B)]

    in_maps = []
    for c in range(8):
        b, hh = c // 2, c % 2
        hs = slice(hh * HD, (hh + 1) * HD)
        in_maps.append({
            "xq": xq_b[b],
            "xk": xk_b[b],
            "xv": xv_b[b],
            "wq": np.ascontiguousarray(wq_eff[:, hs]),
            "wk": np.ascontiguousarray(wk_b[:, hs]),
            "wv": np.ascontiguousarray(wv_b[:, hs]),
            "wo": np.ascontiguousarray(wo_b[hs, :]),
            "bqr": np.ascontiguousarray(bq[hs].reshape(FT, 128).T),
            "bkr": np.ascontiguousarray(bk[hs].reshape(FT, 128).T),
            "bv": bv[None, hs],
        })

    _LAST_IN_MAPS = in_maps
    res = run_bass_kernel_spmd(nc, in_maps, core_ids=list(range(8)))

    out = np.empty((B, S, D), dtype=np.float32)
    for b in range(B):
        out[b] = res.results[2 * b]["o"] + res.results[2 * b + 1]["o"] + bo
    return out


# revision 17
# speedup vs baseline: 1.1079x; 1.1079x over previous
"""Multi-headed attention (B=4, S=2048, D=1024, H=16) on 8 trn2 NeuronCores.

Sharding: core c handles batch b=c//2, head-half hh=c%2 (heads hh*8..hh*8+7).

v2: single fused pipeline (no phase barrier), all inputs pre-cast to bf16 on
the host (no on-device casts), V bias applied on DVE during drain (no K=1
bias matmuls), and background PE work (V projection, previous-tile output
projection) interleaved into the scores stream so the ACT engine (softmax
exp, the critical resource at ~285us) never starves.

Per core:
  K projection (feature-major [512, 2048]) emitted first; then per query
  tile t (512 queries) and head-pair j: Q projection, scores_T via paired
  K=64 matmuls on PE row halves, exp on ACT (1/8 scale folded), AV with
  ones-augmented V accumulating unnormalized X + row sums.  V-projection
  groups are pumped into the (t=0, j=0) scores stream; output-projection
  groups for tile t-1 are pumped into tile t's scores stream.
Host: out[b] = core(2b) + core(2b+1) + bo.
"""

import numpy as np
import ml_dtypes

import concourse.tile as tile
from concourse import bacc, mybir
from concourse.bass_utils import run_bass_kernel_spmd

B, S, D, H = 4, 2048, 1024, 16
HD = D // 2          # feature columns per core (8 heads * 64)
KC = D // 128        # 8 contraction chunks over model dim
FT = HD // 128       # 4 feature tiles (head pairs)
ST = S // 512        # 4 query tiles
RT = S // 128        # 16 row tiles / S_k chunks

f32 = mybir.dt.float32
bf16 = mybir.dt.bfloat16
MM_DT = bf16
EXP = mybir.ActivationFunctionType.Exp

_CACHED_NC = None
_LAST_IN_MAPS = None


def build_nc():
    nc = bacc.Bacc("TRN2", target_bir_lowering=False, debug=False)

    xq_d = nc.dram_tensor("xq", (D, S), bf16, kind="ExternalInput")
    xk_d = nc.dram_tensor("xk", (D, S), bf16, kind="ExternalInput")
    xv_d = nc.dram_tensor("xv", (D, S), bf16, kind="ExternalInput")
    wq_d = nc.dram_tensor("wq", (D, HD), bf16, kind="ExternalInput")
    wk_d = nc.dram_tensor("wk", (D, HD), bf16, kind="ExternalInput")
    wv_d = nc.dram_tensor("wv", (D, HD), bf16, kind="ExternalInput")
    wo_d = nc.dram_tensor("wo", (HD, D), bf16, kind="ExternalInput")
    bqr_d = nc.dram_tensor("bqr", (128, FT), f32, kind="ExternalInput")
    bkr_d = nc.dram_tensor("bkr", (128, FT), f32, kind="ExternalInput")
    bv_d = nc.dram_tensor("bv", (1, HD), f32, kind="ExternalInput")
    o_d = nc.dram_tensor("o", (S, D), f32, kind="ExternalOutput")

    with tile.TileContext(nc) as tc:
        with (
            tc.tile_pool(name="cpool", bufs=1) as cpool,
            tc.tile_pool(name="big", bufs=1) as big,
            tc.tile_pool(name="xs", bufs=40) as xsp,
            tc.tile_pool(name="xqp", bufs=16) as xqp,
            tc.tile_pool(name="qt", bufs=3) as qtp,
            tc.tile_pool(name="ptp", bufs=16) as ptp,
            tc.tile_pool(name="nrm", bufs=4) as nrm,
            tc.tile_pool(name="rsp", bufs=2) as rsp,
            tc.tile_pool(name="ostage", bufs=4) as ostage,
            tc.tile_pool(name="rsd", bufs=2, space="DRAM") as rsd,
            tc.tile_pool(name="psc", bufs=2, space="PSUM") as psc,
            tc.tile_pool(name="px", bufs=2, space="PSUM") as px,
            tc.tile_pool(name="pq", bufs=2, space="PSUM") as pq,
        ):
            # ---------------- constants / biases ----------------
            onecol_f = cpool.tile([128, 1], f32, name="onecol_f")
            nc.gpsimd.memset(onecol_f[:], 1.0)
            bqr_s = cpool.tile([128, FT], f32, name="bqr_s")
            nc.sync.dma_start(bqr_s[:], bqr_d[:])
            bkr_s = cpool.tile([128, FT], f32, name="bkr_s")
            nc.sync.dma_start(bkr_s[:], bkr_d[:])
            bv_bc = cpool.tile([128, HD], f32, name="bv_bc")
            nc.sync.dma_start(bv_bc[:], bv_d[0:1, :].to_broadcast((128, HD)))

            K = big.tile([128, FT, S], MM_DT, name="Kfm")
            Vs = big.tile([128, RT, 8, 65], MM_DT, name="Vs")
            X = big.tile([128, FT, S], MM_DT, name="Xfm")
            nc.vector.tensor_copy(
                Vs[:, :, :, 64:65],
                onecol_f[:, 0:1].to_broadcast((128, RT, 8, 1)),
            )

            # ---------------- weights: direct bf16 DMA ----------------
            wk_s = big.tile([128, KC, HD], MM_DT, name="wk_s")
            wv_s = big.tile([128, KC, HD], MM_DT, name="wv_s")
            wq_s = big.tile([128, KC, HD], MM_DT, name="wq_s")
            wo_s = big.tile([128, FT, D], MM_DT, name="wo_s")
            wk_src = wk_d[:].rearrange("(k p) n -> p k n", p=128)
            wv_src = wv_d[:].rearrange("(k p) n -> p k n", p=128)
            wq_src = wq_d[:].rearrange("(k p) n -> p k n", p=128)
            wo_src = wo_d[:].rearrange("(k p) n -> p k n", p=128)
            for kc in range(KC):
                nc.sync.dma_start(wk_s[:, kc, :], wk_src[:, kc, :])
            for kc in range(KC):
                nc.gpsimd.dma_start(wv_s[:, kc, :], wv_src[:, kc, :])
            for kc in range(KC):
                nc.gpsimd.dma_start(wq_s[:, kc, :], wq_src[:, kc, :])
            for fc in range(FT):
                nc.gpsimd.dma_start(wo_s[:, fc, :], wo_src[:, fc, :])

            def load_x_tiles(x_d, t, eng, tag):
                ts = []
                for kc in range(KC):
                    xt = xsp.tile([128, 512], MM_DT, tag=tag, name=tag)
                    eng.dma_start(
                        xt[:], x_d[kc * 128 : (kc + 1) * 128,
                                   t * 512 : (t + 1) * 512])
                    ts.append(xt)
                return ts

            # ---------------- K projection (ft-major for early scores) ----
            xq_tiles = {}

            def load_xq(t):
                xq_tiles[t] = []
                for kc in range(KC):
                    xt = xqp.tile([128, 512], MM_DT, tag="xq", name="xq")
                    nc.gpsimd.dma_start(
                        xt[:], xq_d[kc * 128 : (kc + 1) * 128,
                                    t * 512 : (t + 1) * 512])
                    xq_tiles[t].append(xt)

            load_xq(0)
            xk_sets = [load_x_tiles(xk_d, t, nc.sync, "xs")
                       for t in range(ST)]

            def emit_kgroup(ft, t):
                ps = pq.tile([128, 512], f32, tag="pacc", name="pk")
                for kc in range(KC):
                    nc.tensor.matmul(
                        ps[:],
                        wk_s[:, kc, ft * 128 : (ft + 1) * 128],
                        xk_sets[t][kc][:],
                        start=(kc == 0),
                        stop=(kc == KC - 1),
                    )
                nc.vector.tensor_scalar_add(
                    K[:, ft, t * 512 : (t + 1) * 512],
                    ps[:],
                    bkr_s[:, ft : ft + 1],
                )

            # only the j=0 row block before attention; rest pumped into t=0
            for t in range(ST):
                emit_kgroup(0, t)
            bgk = [(ft, t) for ft in range(1, FT) for t in range(ST)]

            # ---------------- deferred V projection groups ----------------
            xv_tiles = {}

            def load_xv(g):
                xv_tiles[g] = load_x_tiles(xv_d, g, nc.gpsimd, "xs")

            load_xv(0)
            vstate = [0]

            def emit_vgroup():
                rt = vstate[0]
                g, rr = rt // 4, rt % 4
                if rr == 0 and g + 1 < 4:
                    load_xv(g + 1)
                ps = pq.tile([128, 512], f32, tag="pacc", name="pv")
                for kc in range(KC):
                    nc.tensor.matmul(
                        ps[:],
                        xv_tiles[g][kc][:, rr * 128 : (rr + 1) * 128],
                        wv_s[:, kc, :],
                        start=(kc == 0),
                        stop=(kc == KC - 1),
                    )
                nc.vector.tensor_add(
                    Vs[:, rt, :, 0:64],
                    ps[:].rearrange("p (h e) -> p h e", h=8),
                    bv_bc[:].rearrange("p (h e) -> p h e", h=8),
                )
                vstate[0] = rt + 1

            ktoggle = [True]

            def pump_v(n):
                # alternate deferred K-row groups with V groups so both
                # complete across t=0 while AV consumption starts early
                for _ in range(n):
                    if ktoggle[0] and bgk:
                        emit_kgroup(*bgk.pop(0))
                    elif vstate[0] < RT:
                        emit_vgroup()
                    elif bgk:
                        emit_kgroup(*bgk.pop(0))
                    ktoggle[0] = not ktoggle[0]

            # ---------------- deferred output projection ----------------
            bg = []

            def mk_outproj(t2):
                def mk(r2, n):
                    def g():
                        rt = t2 * 4 + r2
                        rsl = slice(rt * 128, (rt + 1) * 128)
                        ps = pq.tile([128, 512], f32, tag="pacc", name="pso")
                        for fc in range(FT):
                            nc.tensor.matmul(
                                ps[:],
                                X[:, fc, rsl],
                                wo_s[:, fc, n * 512 : (n + 1) * 512],
                                start=(fc == 0),
                                stop=(fc == FT - 1),
                            )
                        ot = ostage.tile([128, 512], f32, tag="os", name="os")
                        nc.vector.tensor_copy(ot[:], ps[:])
                        oeng = nc.gpsimd if (r2 * 2 + n) % 2 else nc.sync
                        oeng.dma_start(
                            o_d[rsl, n * 512 : (n + 1) * 512], ot[:])
                    return g
                return [mk(r2, n) for r2 in range(4) for n in range(2)]

            def pump_bg(n):
                for _ in range(n):
                    if bg:
                        bg.pop(0)()

            # ---------------- normalization ----------------
            def emit_normalize(j2, rsj, tsl2):
                rrh = nrm.tile([128, 512], f32, tag="rr", name="rr")
                nc.vector.reciprocal_approx_fast(rrh[:], rsj[:])
                rd = rsd.tile([2, 512], f32, tag="rd", name="rd")
                for hh in range(2):
                    nc.sync.dma_start(
                        rd[hh : hh + 1, :],
                        rrh[32 * hh : 32 * hh + 1, :])
                for hh in range(2):
                    pb = 64 * hh
                    bcs = nrm.tile([128, 512], f32, tag="bcs", name="bcs")
                    nc.sync.dma_start(
                        bcs[pb : pb + 64, :],
                        rd[hh : hh + 1, :].to_broadcast((64, 512)))
                    nc.vector.tensor_mul(
                        X[pb : pb + 64, j2, tsl2],
                        X[pb : pb + 64, j2, tsl2],
                        bcs[pb : pb + 64, :],
                    )

            # ---------------- attention ----------------
            def emit_av(item, xpA, xpB, j):
                cc, pA, pB = item
                for hf in range(2):
                    kc = 2 * cc + hf
                    nc.tensor.matmul(
                        xpA[:], Vs[:, kc, 2 * j, :], pA[:, hf, :],
                        start=(kc == 0), stop=(kc == RT - 1),
                    )
                    nc.tensor.matmul(
                        xpB[:], Vs[:, kc, 2 * j + 1, :], pB[:, hf, :],
                        start=(kc == 0), stop=(kc == RT - 1),
                    )

            norm_pending = None
            for t in range(ST):
                tsl = slice(t * 512, (t + 1) * 512)
                for j in range(FT):
                    first = (t == 0)
                    # Q projection for this pair
                    qp = pq.tile([128, 512], f32, tag="pacc", name="qp")
                    for kc in range(KC):
                        nc.tensor.matmul(
                            qp[:],
                            wq_s[:, kc, j * 128 : (j + 1) * 128],
                            xq_tiles[t][kc][:],
                            start=(kc == 0),
                            stop=(kc == KC - 1),
                        )
                    Qt = qtp.tile([128, 512], MM_DT, tag="qt", name="qt")
                    nc.vector.tensor_scalar_add(
                        Qt[:], qp[:], bqr_s[:, j : j + 1])
                    if norm_pending is not None:
                        emit_normalize(*norm_pending)
                        norm_pending = None
                    if j == 2 and t + 1 < ST:
                        load_xq(t + 1)
                    if t == 0:
                        # pair row j's K columns must exist before scores
                        while bgk and bgk[0][0] <= j:
                            emit_kgroup(*bgk.pop(0))

                    xpA = px.tile([65, 512], f32, tag="px", name="xpA")
                    xpB = px.tile([65, 512], f32, tag="px", name="xpB")
                    pend = []
                    for cc in range(8):
                        sA = psc.tile([128, 2, 512], f32, tag="sc", name="sA")
                        sB = psc.tile([128, 2, 512], f32, tag="sc", name="sB")
                        for hf in range(2):
                            kc = 2 * cc + hf
                            ksl = slice(kc * 128, (kc + 1) * 128)
                            nc.tensor.matmul(
                                sA[:, hf, :], K[0:64, j, ksl], Qt[0:64, :],
                                start=True, stop=True, tile_position=(0, 0),
                            )
                            nc.tensor.matmul(
                                sB[:, hf, :], K[64:128, j, ksl],
                                Qt[64:128, :],
                                start=True, stop=True, tile_position=(64, 0),
                            )
                        pA = ptp.tile([128, 2, 512], MM_DT, tag="pt",
                                      name="pA")
                        nc.scalar.activation(pA[:], sA[:], EXP, scale=0.125)
                        pB = ptp.tile([128, 2, 512], MM_DT, tag="pt",
                                      name="pB")
                        nc.scalar.activation(pB[:], sB[:], EXP, scale=0.125)
                        pend.append((cc, pA, pB))
                        if first:
                            # build V while scores of the first pair stream
                            pump_v(2 if cc < 4 else 1)
                            # start AV for chunks whose V rows are ready
                            while (pend and
                                   2 * pend[0][0] + 1 < vstate[0] - 2):
                                emit_av(pend.pop(0), xpA, xpB, j)
                        else:
                            if len(pend) > 2:
                                emit_av(pend.pop(0), xpA, xpB, j)
                            if cc in (2, 4, 6):
                                pump_bg(1)
                    for item in pend:
                        if first:
                            while vstate[0] <= 2 * item[0] + 1:
                                emit_vgroup()
                        emit_av(item, xpA, xpB, j)

                    # drain: unnormalized X and row sums to SBUF
                    nc.vector.tensor_copy(X[0:64, j, tsl], xpA[0:64, :])
                    nc.vector.tensor_copy(X[64:128, j, tsl], xpB[0:64, :])
                    rsj = rsp.tile([128, 512], f32, tag="rs", name="rs")
                    nc.vector.tensor_copy(rsj[0:1, :], xpA[64:65, :])
                    nc.vector.tensor_copy(rsj[32:33, :], xpB[64:65, :])
                    norm_pending = (j, rsj, tsl)

                # end of tile t: queue its output projection for overlap
                if t == ST - 1:
                    if norm_pending is not None:
                        emit_normalize(*norm_pending)
                        norm_pending = None
                    pump_bg(len(bg))
                    for g in mk_outproj(t):
                        g()
                else:
                    bg.extend(mk_outproj(t))

    nc.compile()
    return nc


def kernel(**inputs):
    global _CACHED_NC, _LAST_IN_MAPS
    if _CACHED_NC is None:
        _CACHED_NC = build_nc()
    nc = _CACHED_NC

    bfdt = ml_dtypes.bfloat16
    query = np.asarray(inputs["query"], dtype=np.float32)
    key = np.asarray(inputs["key"], dtype=np.float32)
    value = np.asarray(inputs["value"], dtype=np.float32)
    fc_w = np.asarray(inputs["fc_w"], dtype=np.float32)
    Wq = np.asarray(inputs["Wq"], dtype=np.float32)
    Wk = np.asarray(inputs["Wk"], dtype=np.float32)
    Wv = np.asarray(inputs["Wv"], dtype=np.float32)
    Wo = np.asarray(inputs["Wo"], dtype=np.float32)
    bq = np.asarray(inputs["bq"], dtype=np.float32)
    bk = np.asarray(inputs["bk"], dtype=np.float32)
    bv = np.asarray(inputs["bv"], dtype=np.float32)
    bo = np.asarray(inputs["bo"], dtype=np.float32)

    wq_eff = (fc_w * Wq).astype(bfdt)
    wk_b = Wk.astype(bfdt)
    wv_b = Wv.astype(bfdt)
    wo_b = Wo.astype(bfdt)
    xq_b = [np.ascontiguousarray(query[b].T).astype(bfdt) for b in range(B)]
    xk_b = [np.ascontiguousarray(key[b].T).astype(bfdt) for b in range(B)]
    xv_b = [np.ascontiguousarray(value[b].T).astype(bfdt) for b in range(B)]

    in_maps = []
    for c in range(8):
        b, hh = c // 2, c % 2
        hs = slice(hh * HD, (hh + 1) * HD)
        in_maps.append({
            "xq": xq_b[b],
            "xk": xk_b[b],
            "xv": xv_b[b],
            "wq": np.ascontiguousarray(wq_eff[:, hs]),
            "wk": np.ascontiguousarray(wk_b[:, hs]),
            "wv": np.ascontiguousarray(wv_b[:, hs]),
            "wo": np.ascontiguousarray(wo_b[hs, :]),
            "bqr": np.ascontiguousarray(bq[hs].reshape(FT, 128).T),
            "bkr": np.ascontiguousarray(bk[hs].reshape(FT, 128).T),
            "bv": bv[None, hs],
        })

    _LAST_IN_MAPS = in_maps
    res = run_bass_kernel_spmd(nc, in_maps, core_ids=list(range(8)))

    out = np.empty((B, S, D), dtype=np.float32)
    for b in range(B):
        out[b] = res.results[2 * b]["o"] + res.results[2 * b + 1]["o"] + bo
    return out


# revision 19
# speedup vs baseline: 1.1340x; 1.0236x over previous
"""Multi-headed attention (B=4, S=2048, D=1024, H=16) on 8 trn2 NeuronCores.

Sharding: core c handles batch b=c//2, head-half hh=c%2 (heads hh*8..hh*8+7).

v2: single fused pipeline (no phase barrier), all inputs pre-cast to bf16 on
the host (no on-device casts), V bias applied on DVE during drain (no K=1
bias matmuls), and background PE work (V projection, previous-tile output
projection) interleaved into the scores stream so the ACT engine (softmax
exp, the critical resource at ~285us) never starves.

Per core:
  K projection (feature-major [512, 2048]) emitted first; then per query
  tile t (512 queries) and head-pair j: Q projection, scores_T via paired
  K=64 matmuls on PE row halves, exp on ACT (1/8 scale folded), AV with
  ones-augmented V accumulating unnormalized X + row sums.  V-projection
  groups are pumped into the (t=0, j=0) scores stream; output-projection
  groups for tile t-1 are pumped into tile t's scores stream.
Host: out[b] = core(2b) + core(2b+1) + bo.
"""

import numpy as np
import ml_dtypes

import concourse.tile as tile
from concourse import bacc, mybir
from concourse.bass_utils import run_bass_kernel_spmd

B, S, D, H = 4, 2048, 1024, 16
HD = D // 2          # feature columns per core (8 heads * 64)
KC = D // 128        # 8 contraction chunks over model dim
FT = HD // 128       # 4 feature tiles (head pairs)
ST = S // 512        # 4 query tiles
RT = S // 128        # 16 row tiles / S_k chunks

f32 = mybir.dt.float32
bf16 = mybir.dt.bfloat16
MM_DT = bf16
EXP = mybir.ActivationFunctionType.Exp

_CACHED_NC = None
_LAST_IN_MAPS = None


def build_nc():
    nc = bacc.Bacc("TRN2", target_bir_lowering=False, debug=False)

    xq_d = nc.dram_tensor("xq", (D, S), bf16, kind="ExternalInput")
    xk_d = nc.dram_tensor("xk", (D, S), bf16, kind="ExternalInput")
    xv_d = nc.dram_tensor("xv", (D, S), bf16, kind="ExternalInput")
    wq_d = nc.dram_tensor("wq", (D, HD), bf16, kind="ExternalInput")
    wk_d = nc.dram_tensor("wk", (D, HD), bf16, kind="ExternalInput")
    wv_d = nc.dram_tensor("wv", (D, HD), bf16, kind="ExternalInput")
    wo_d = nc.dram_tensor("wo", (HD, D), bf16, kind="ExternalInput")
    bqr_d = nc.dram_tensor("bqr", (128, FT), f32, kind="ExternalInput")
    bkr_d = nc.dram_tensor("bkr", (128, FT), f32, kind="ExternalInput")
    bv_d = nc.dram_tensor("bv", (1, HD), f32, kind="ExternalInput")
    o_d = nc.dram_tensor("o", (S, D), f32, kind="ExternalOutput")

    with tile.TileContext(nc) as tc:
        with (
            tc.tile_pool(name="cpool", bufs=1) as cpool,
            tc.tile_pool(name="big", bufs=1) as big,
            tc.tile_pool(name="xs", bufs=40) as xsp,
            tc.tile_pool(name="xqp", bufs=16) as xqp,
            tc.tile_pool(name="qt", bufs=3) as qtp,
            tc.tile_pool(name="ptp", bufs=16) as ptp,
            tc.tile_pool(name="nrm", bufs=4) as nrm,
            tc.tile_pool(name="rsp", bufs=2) as rsp,
            tc.tile_pool(name="ostage", bufs=4) as ostage,
            tc.tile_pool(name="rsd", bufs=2, space="DRAM") as rsd,
            tc.tile_pool(name="psc", bufs=2, space="PSUM") as psc,
            tc.tile_pool(name="px", bufs=2, space="PSUM") as px,
            tc.tile_pool(name="pq", bufs=2, space="PSUM") as pq,
        ):
            # ---------------- constants / biases ----------------
            onecol_f = cpool.tile([128, 1], f32, name="onecol_f")
            nc.gpsimd.memset(onecol_f[:], 1.0)
            bqr_s = cpool.tile([128, FT], f32, name="bqr_s")
            nc.sync.dma_start(bqr_s[:], bqr_d[:])
            bkr_s = cpool.tile([128, FT], f32, name="bkr_s")
            nc.sync.dma_start(bkr_s[:], bkr_d[:])
            bv_bc = cpool.tile([128, HD], f32, name="bv_bc")
            nc.sync.dma_start(bv_bc[:], bv_d[0:1, :].to_broadcast((128, HD)))

            K = big.tile([128, FT, S], MM_DT, name="Kfm")
            Vs = big.tile([128, RT, 8, 65], MM_DT, name="Vs")
            X = big.tile([128, FT, S], MM_DT, name="Xfm")
            nc.vector.tensor_copy(
                Vs[:, :, :, 64:65],
                onecol_f[:, 0:1].to_broadcast((128, RT, 8, 1)),
            )

            # ---------------- weights: direct bf16 DMA ----------------
            wk_s = big.tile([128, KC, HD], MM_DT, name="wk_s")
            wv_s = big.tile([128, KC, HD], MM_DT, name="wv_s")
            wq_s = big.tile([128, KC, HD], MM_DT, name="wq_s")
            wo_s = big.tile([128, FT, D], MM_DT, name="wo_s")
            wk_src = wk_d[:].rearrange("(k p) n -> p k n", p=128)
            wv_src = wv_d[:].rearrange("(k p) n -> p k n", p=128)
            wq_src = wq_d[:].rearrange("(k p) n -> p k n", p=128)
            wo_src = wo_d[:].rearrange("(k p) n -> p k n", p=128)
            for kc in range(KC):
                nc.sync.dma_start(wk_s[:, kc, :], wk_src[:, kc, :])
            for kc in range(KC):
                nc.gpsimd.dma_start(wv_s[:, kc, :], wv_src[:, kc, :])
            for kc in range(KC):
                nc.gpsimd.dma_start(wq_s[:, kc, :], wq_src[:, kc, :])
            for fc in range(FT):
                nc.gpsimd.dma_start(wo_s[:, fc, :], wo_src[:, fc, :])

            def load_x_tiles(x_d, t, eng, tag):
                ts = []
                for kc in range(KC):
                    xt = xsp.tile([128, 512], MM_DT, tag=tag, name=tag)
                    eng.dma_start(
                        xt[:], x_d[kc * 128 : (kc + 1) * 128,
                                   t * 512 : (t + 1) * 512])
                    ts.append(xt)
                return ts

            # ---------------- K projection (ft-major for early scores) ----
            xq_tiles = {}

            def load_xq(t):
                xq_tiles[t] = []
                for kc in range(KC):
                    xt = xqp.tile([128, 512], MM_DT, tag="xq", name="xq")
                    nc.gpsimd.dma_start(
                        xt[:], xq_d[kc * 128 : (kc + 1) * 128,
                                    t * 512 : (t + 1) * 512])
                    xq_tiles[t].append(xt)

            load_xq(0)
            xk_sets = [load_x_tiles(xk_d, t, nc.sync, "xs")
                       for t in range(ST)]

            def emit_kgroup(ft, t):
                ps = pq.tile([128, 512], f32, tag="pacc", name="pk")
                for kc in range(KC):
                    nc.tensor.matmul(
                        ps[:],
                        wk_s[:, kc, ft * 128 : (ft + 1) * 128],
                        xk_sets[t][kc][:],
                        start=(kc == 0),
                        stop=(kc == KC - 1),
                    )
                nc.vector.tensor_scalar_add(
                    K[:, ft, t * 512 : (t + 1) * 512],
                    ps[:],
                    bkr_s[:, ft : ft + 1],
                )

            # only the j=0 row block before attention; rest pumped into t=0
            for t in range(ST):
                emit_kgroup(0, t)
            bgk = [(ft, t) for ft in range(1, FT) for t in range(ST)]

            # ---------------- deferred V projection groups ----------------
            xv_tiles = {}

            def load_xv(g):
                xv_tiles[g] = load_x_tiles(xv_d, g, nc.gpsimd, "xs")

            load_xv(0)
            vstate = [0]

            def emit_vgroup():
                rt = vstate[0]
                g, rr = rt // 4, rt % 4
                if rr == 0 and g + 1 < 4:
                    load_xv(g + 1)
                ps = pq.tile([128, 512], f32, tag="pacc", name="pv")
                for kc in range(KC):
                    nc.tensor.matmul(
                        ps[:],
                        xv_tiles[g][kc][:, rr * 128 : (rr + 1) * 128],
                        wv_s[:, kc, :],
                        start=(kc == 0),
                        stop=(kc == KC - 1),
                    )
                nc.vector.tensor_add(
                    Vs[:, rt, :, 0:64],
                    ps[:].rearrange("p (h e) -> p h e", h=8),
                    bv_bc[:].rearrange("p (h e) -> p h e", h=8),
                )
                vstate[0] = rt + 1

            ktoggle = [True]

            def pump_v(n):
                # alternate deferred K-row groups with V groups so both
                # complete across t=0 while AV consumption starts early
                for _ in range(n):
                    if ktoggle[0] and bgk:
                        emit_kgroup(*bgk.pop(0))
                    elif vstate[0] < RT:
                        emit_vgroup()
                    elif bgk:
                        emit_kgroup(*bgk.pop(0))
                    ktoggle[0] = not ktoggle[0]

            # ---------------- deferred output projection ----------------
            bg = []

            def mk_outproj(t2):
                def mk(r2, n):
                    def g():
                        rt = t2 * 4 + r2
                        rsl = slice(rt * 128, (rt + 1) * 128)
                        ps = pq.tile([128, 512], f32, tag="pacc", name="pso")
                        for fc in range(FT):
                            nc.tensor.matmul(
                                ps[:],
                                X[:, fc, rsl],
                                wo_s[:, fc, n * 512 : (n + 1) * 512],
                                start=(fc == 0),
                                stop=(fc == FT - 1),
                            )
                        ot = ostage.tile([128, 512], f32, tag="os", name="os")
                        nc.vector.tensor_copy(ot[:], ps[:])
                        oeng = nc.gpsimd if (r2 * 2 + n) % 2 else nc.sync
                        oeng.dma_start(
                            o_d[rsl, n * 512 : (n + 1) * 512], ot[:])
                    return g
                return [mk(r2, n) for r2 in range(4) for n in range(2)]

            def pump_bg(n):
                for _ in range(n):
                    if bg:
                        bg.pop(0)()

            # ---------------- normalization ----------------
            def emit_normalize(j2, rsj, tsl2):
                rrh = nrm.tile([128, 512], f32, tag="rr", name="rr")
                nc.vector.reciprocal_approx_fast(rrh[:], rsj[:])
                rd = rsd.tile([2, 512], f32, tag="rd", name="rd")
                for hh in range(2):
                    nc.sync.dma_start(
                        rd[hh : hh + 1, :],
                        rrh[32 * hh : 32 * hh + 1, :])
                for hh in range(2):
                    pb = 64 * hh
                    bcs = nrm.tile([128, 512], f32, tag="bcs", name="bcs")
                    nc.sync.dma_start(
                        bcs[pb : pb + 64, :],
                        rd[hh : hh + 1, :].to_broadcast((64, 512)))
                    nc.vector.tensor_mul(
                        X[pb : pb + 64, j2, tsl2],
                        X[pb : pb + 64, j2, tsl2],
                        bcs[pb : pb + 64, :],
                    )

            # ---------------- attention ----------------
            def emit_av(item, xpA, xpB, j):
                cc, pA, pB = item
                for hf in range(2):
                    kc = 2 * cc + hf
                    nc.tensor.matmul(
                        xpA[:], Vs[:, kc, 2 * j, :], pA[:, hf, :],
                        start=(kc == 0), stop=(kc == RT - 1),
                    )
                    nc.tensor.matmul(
                        xpB[:], Vs[:, kc, 2 * j + 1, :], pB[:, hf, :],
                        start=(kc == 0), stop=(kc == RT - 1),
                    )

            def emit_qt(t2, j2):
                qp = pq.tile([128, 512], f32, tag="pacc", name="qp")
                for kc in range(KC):
                    nc.tensor.matmul(
                        qp[:],
                        wq_s[:, kc, j2 * 128 : (j2 + 1) * 128],
                        xq_tiles[t2][kc][:],
                        start=(kc == 0),
                        stop=(kc == KC - 1),
                    )
                Qt = qtp.tile([128, 512], MM_DT, tag="qt", name="qt")
                nc.vector.tensor_scalar_add(
                    Qt[:], qp[:], bqr_s[:, j2 : j2 + 1])
                return Qt

            norm_pending = None
            qt_next = emit_qt(0, 0)
            for t in range(ST):
                tsl = slice(t * 512, (t + 1) * 512)
                for j in range(FT):
                    first = (t == 0)
                    Qt = qt_next
                    if norm_pending is not None:
                        emit_normalize(*norm_pending)
                        norm_pending = None
                    if j == 2 and t + 1 < ST:
                        load_xq(t + 1)
                    if t == 0:
                        # pair row j's K columns must exist before scores
                        while bgk and bgk[0][0] <= j:
                            emit_kgroup(*bgk.pop(0))

                    xpA = px.tile([65, 512], f32, tag="px", name="xpA")
                    xpB = px.tile([65, 512], f32, tag="px", name="xpB")
                    pend = []
                    for cc in range(8):
                        sA = psc.tile([128, 2, 512], f32, tag="sc", name="sA")
                        sB = psc.tile([128, 2, 512], f32, tag="sc", name="sB")
                        for hf in range(2):
                            kc = 2 * cc + hf
                            ksl = slice(kc * 128, (kc + 1) * 128)
                            nc.tensor.matmul(
                                sA[:, hf, :], K[0:64, j, ksl], Qt[0:64, :],
                                start=True, stop=True, tile_position=(0, 0),
                            )
                            nc.tensor.matmul(
                                sB[:, hf, :], K[64:128, j, ksl],
                                Qt[64:128, :],
                                start=True, stop=True, tile_position=(64, 0),
                            )
                        pA = ptp.tile([128, 2, 512], MM_DT, tag="pt",
                                      name="pA")
                        nc.scalar.activation(pA[:], sA[:], EXP, scale=0.125)
                        pB = ptp.tile([128, 2, 512], MM_DT, tag="pt",
                                      name="pB")
                        nc.scalar.activation(pB[:], sB[:], EXP, scale=0.125)
                        pend.append((cc, pA, pB))
                        if cc == 5:
                            # project next section's Q mid-section so its
                            # scores start right after this section's AV
                            if j + 1 < FT:
                                qt_next = emit_qt(t, j + 1)
                            elif t + 1 < ST:
                                qt_next = emit_qt(t + 1, 0)
                        if first:
                            # build V while scores of the first pair stream
                            pump_v(2 if cc < 4 else 1)
                            # start AV for chunks whose V rows are ready
                            while (pend and
                                   2 * pend[0][0] + 1 < vstate[0] - 2):
                                emit_av(pend.pop(0), xpA, xpB, j)
                        else:
                            if len(pend) > 2:
                                emit_av(pend.pop(0), xpA, xpB, j)
                            if cc in (2, 4, 6):
                                pump_bg(1)
                    for item in pend:
                        if first:
                            while vstate[0] <= 2 * item[0] + 1:
                                emit_vgroup()
                        emit_av(item, xpA, xpB, j)

                    # drain: unnormalized X and row sums to SBUF
                    nc.vector.tensor_copy(X[0:64, j, tsl], xpA[0:64, :])
                    nc.vector.tensor_copy(X[64:128, j, tsl], xpB[0:64, :])
                    rsj = rsp.tile([128, 512], f32, tag="rs", name="rs")
                    nc.vector.tensor_copy(rsj[0:1, :], xpA[64:65, :])
                    nc.vector.tensor_copy(rsj[32:33, :], xpB[64:65, :])
                    norm_pending = (j, rsj, tsl)

                # end of tile t: queue its output projection for overlap
                if t == ST - 1:
                    if norm_pending is not None:
                        emit_normalize(*norm_pending)
                        norm_pending = None
                    pump_bg(len(bg))
                    for g in mk_outproj(t):
                        g()
                else:
                    bg.extend(mk_outproj(t))

    nc.compile()
    return nc


def kernel(**inputs):
    global _CACHED_NC, _LAST_IN_MAPS
    if _CACHED_NC is None:
        _CACHED_NC = build_nc()
    nc = _CACHED_NC

    bfdt = ml_dtypes.bfloat16
    query = np.asarray(inputs["query"], dtype=np.float32)
    key = np.asarray(inputs["key"], dtype=np.float32)
    value = np.asarray(inputs["value"], dtype=np.float32)
    fc_w = np.asarray(inputs["fc_w"], dtype=np.float32)
    Wq = np.asarray(inputs["Wq"], dtype=np.float32)
    Wk = np.asarray(inputs["Wk"], dtype=np.float32)
    Wv = np.asarray(inputs["Wv"], dtype=np.float32)
    Wo = np.asarray(inputs["Wo"], dtype=np.float32)
    bq = np.asarray(inputs["bq"], dtype=np.float32)
    bk = np.asarray(inputs["bk"], dtype=np.float32)
    bv = np.asarray(inputs["bv"], dtype=np.float32)
    bo = np.asarray(inputs["bo"], dtype=np.float32)

    wq_eff = (fc_w * Wq).astype(bfdt)
    wk_b = Wk.astype(bfdt)
    wv_b = Wv.astype(bfdt)
    wo_b = Wo.astype(bfdt)
    xq_b = [np.ascontiguousarray(query[b].T).astype(bfdt) for b in range(B)]
    xk_b = [np.ascontiguousarray(key[b].T).astype(bfdt) for b in range(B)]
    xv_b = [np.ascontiguousarray(value[b].T).astype(bfdt) for b in range(B)]

    in_maps = []
    for c in range(8):
        b, hh = c // 2, c % 2
        hs = slice(hh * HD, (hh + 1) * HD)
        in_maps.append({
            "xq": xq_b[b],
            "xk": xk_b[b],
            "xv": xv_b[b],
            "wq": np.ascontiguousarray(wq_eff[:, hs]),
            "wk": np.ascontiguousarray(wk_b[:, hs]),
            "wv": np.ascontiguousarray(wv_b[:, hs]),
            "wo": np.ascontiguousarray(wo_b[hs, :]),
            "bqr": np.ascontiguousarray(bq[hs].reshape(FT, 128).T),
            "bkr": np.ascontiguousarray(bk[hs].reshape(FT, 128).T),
            "bv": bv[None, hs],
        })

    _LAST_IN_MAPS = in_maps
    res = run_bass_kernel_spmd(nc, in_maps, core_ids=list(range(8)))

    out = np.empty((B, S, D), dtype=np.float32)
    for b in range(B):
        out[b] = res.results[2 * b]["o"] + res.results[2 * b + 1]["o"] + bo
    return out
